# revision 41
# baseline (speedup 1.0000x reference)
"""Trainium2 Bass kernel for nn_BandedJointEncoder.

Math: the reference's solve_triangular(P, I, lower=True) only reads the
lower triangle of P, which is purely diagonal (the band sits on the
superdiagonal).  So

  scale[b, l, i, j] = delta_ij / (softplus(prec[b, l, i]) + 1)
  prec[b, l, i]     = mapped[b, l*16 + i//32, 16 + (i % 32)]   (torch-style
                      row-major reshape of the [B, T, 2L] softplus input)
  mean[b, l, t]     = mapped[b, t, l]

where mapped = MLP(conv1d(x)).  The 256 MiB `scale` output is diagonal;
per core we keep a persistent zero SBUF tile and only rewrite the 256
diagonal slots per batch, then stream 4 MiB/batch to HBM.

Notes: trn2 engines require partition starts in {0,32,64,96}, so the
diag stripes use j-outer partition order (each stripe = 16 contiguous
partitions) with a per-stripe column shift of 224-32j that makes the
diagonal slots uniform across partitions (one DVE copy per batch); the
8 per-stripe DMAs absorb the shift via their column windows.  Bacc's
compile() legalizes multi-wait instructions; the DVE observer chain and
bufs=8 working tiles just reduce semaphore traffic.  All activations
resolve to the single natural_log_exp_and_others ACT table so only one
LoadActFuncSet is emitted (the default chooser swaps tables twice per
batch).

Sharding: pure data parallel, batch 64 -> 8 cores x 8 batches.
"""

import numpy as np

B, T, D = 64, 256, 64
H = 128
L = 16
KW = 3
NCORES = 8
BPC = B // NCORES  # batches per core

_CACHE = {}


def _build_bass():
    import concourse.bacc as bacc
    import concourse.mybir as mybir
    from concourse.masks import make_identity
    from concourse.tile import TileContext, add_dep_helper

    fp = mybir.dt.float32
    AF = mybir.ActivationFunctionType

    nc = bacc.Bacc()
    x = nc.dram_tensor("x", [BPC, T, D], fp, kind="ExternalInput")
    conv_w = nc.dram_tensor("conv_w", [H, D, KW], fp, kind="ExternalInput")
    conv_b = nc.dram_tensor("conv_b", [H], fp, kind="ExternalInput")
    w1 = nc.dram_tensor("w1", [H, H], fp, kind="ExternalInput")
    b1 = nc.dram_tensor("b1", [H], fp, kind="ExternalInput")
    w2 = nc.dram_tensor("w2", [H, H], fp, kind="ExternalInput")
    b2 = nc.dram_tensor("b2", [H], fp, kind="ExternalInput")
    w3 = nc.dram_tensor("w3", [H, 3 * L], fp, kind="ExternalInput")
    b3 = nc.dram_tensor("b3", [3 * L], fp, kind="ExternalInput")
    mean_o = nc.dram_tensor("mean_o", [BPC, L, T], fp, kind="ExternalOutput")
    scale_o = nc.dram_tensor("scale_o", [BPC, L, T, T], fp, kind="ExternalOutput")

    # scale[b] viewed as [128, 8192]: partition p = j*16 + l holds the
    # 32-row stripe i in [32j, 32j+32) of matrix (b, l); free offset =
    # c*256 + k for stripe row c, col k.  Diagonal of stripe row c is at
    # c*257 + 32j.  j outer => each stripe j is 16 CONTIGUOUS partitions
    # (strided-partition SBUF DMA access patterns are not supported).
    scale_v = scale_o.rearrange("b l (j c) k -> b j l (c k)", j=8)

    # ---- raw preamble: constants + input loads, then barrier ----
    # GPSIMD ops run concurrently across Q7 cores: the memset must be
    # sem-fenced before affine_select reads it.
    ident = nc.alloc_sbuf_tensor("identg", [128, 128], fp).ap()
    id_sem = nc.alloc_semaphore("ident_sem")
    nc.gpsimd.memset(ident, 0.0).then_inc(id_sem, 1)
    e_const = nc.alloc_sbuf_tensor("e_const", [128, 1], fp).ap()
    nc.gpsimd.memset(e_const, 2.718281828459045)
    nc.gpsimd.wait_ge(id_sem, 1)
    make_identity(nc, ident, nomemset=True)

    nc.all_engine_barrier()

    with TileContext(nc) as tc:
        with (
            tc.tile_pool(name="const", bufs=1) as const,
            tc.tile_pool(name="work", bufs=8) as work,
            tc.tile_pool(name="diag", bufs=1) as diagp,
            tc.tile_pool(name="ps", bufs=1, space="PSUM") as psp,
        ):
            # ---- tracked input loads (overlap with memsets/compute) ----
            b3v = b3.rearrange("(c o) -> c o", o=1)
            xall = const.tile([128, BPC, 2, D], fp, tag="xall", name="xall")
            nc.sync.dma_start(xall[:], x.rearrange("b (n p) d -> p b n d", p=128))
            cw_sb = const.tile([H, D * KW], fp, tag="cw", name="cw_sb")
            nc.sync.dma_start(cw_sb[:], conv_w.rearrange("h d k -> h (d k)"))
            w1_sb = const.tile([H, H], fp, tag="w1", name="w1_sb")
            nc.sync.dma_start(w1_sb[:], w1[:])
            w2_sb = const.tile([H, H], fp, tag="w2", name="w2_sb")
            nc.sync.dma_start(w2_sb[:], w2[:])
            w3_sb = const.tile([H, 3 * L], fp, tag="w3", name="w3_sb")
            nc.sync.dma_start(w3_sb[:], w3[:])
            cb_sb = const.tile([H, 1], fp, tag="cb", name="cb_sb")
            nc.sync.dma_start(cb_sb[:], conv_b.rearrange("(h o) -> h o", o=1))
            b1_sb = const.tile([H, 1], fp, tag="b1", name="b1_sb")
            nc.sync.dma_start(b1_sb[:], b1.rearrange("(h o) -> h o", o=1))
            b2_sb = const.tile([H, 1], fp, tag="b2", name="b2_sb")
            nc.sync.dma_start(b2_sb[:], b2.rearrange("(h o) -> h o", o=1))
            b3m_sb = const.tile([L, 1], fp, tag="b3m", name="b3m_sb")
            nc.sync.dma_start(b3m_sb[:], b3v[0:L])
            b3p_sb = const.tile([2 * L, 1], fp, tag="b3p", name="b3p_sb")
            nc.sync.dma_start(b3p_sb[:], b3v[L : 3 * L])
            # conv taps: w_tap[k][d, h] = conv_w[h, d, k]
            cw_v = cw_sb.rearrange("h (d k) -> h k d", k=KW)
            w_tap = []
            for k in range(KW):
                tp = psp.tile([D, H], fp, tag="xt", name="tp")
                nc.tensor.transpose(tp[:], cw_v[:, k, :], ident)
                wt = const.tile([D, H], fp, tag=f"tap{k}", name=f"tap{k}")
                nc.scalar.copy(wt[:], tp[:])
                w_tap.append(wt)

            # Persistent x^T tiles: [d, 1+t] with zero pad cols 0 and T+1
            xTs = [
                const.tile([D, T + 2], fp, tag="xTA", name="xTA"),
                const.tile([D, T + 2], fp, tag="xTB", name="xTB"),
            ]
            for t_ in xTs:
                nc.vector.memset(t_[:, 0:1], 0.0)
                nc.vector.memset(t_[:, T + 1 : T + 2], 0.0)

            # Persistent diagonal-output tiles (zeros except diag slots).
            # Partition p = j*16+l stores the 32-row stripe i in [32j,32j+32)
            # of matrix (b, l) at free offset 224-32j, so every partition's
            # diag slots sit at uniform positions 224 + 257*c.  Each tile
            # holds a PAIR of batches (two SPITCH halves) so each per-stripe
            # DMA moves 1 MiB (2 batches x 16 partitions x 32 KiB).
            SPITCH = 8192 + 224
            NDG = 3
            dg = [
                diagp.tile([128, SPITCH], fp, tag=f"dg{i}", name=f"dg{i}")
                for i in range(NDG)
            ]
            for i, t_ in enumerate(dg):
                (nc.vector if i % 2 == 0 else nc.gpsimd).memset(t_[:], 0.0)

            # per-batch mean staging: one column block per batch, single DMA
            mean_all = const.tile([L, BPC * T], fp, tag="mean_all")

            # ---- per-batch pipeline ----
            prev_diag = None
            last_pack = None
            for b in range(BPC):
                buf = dg[b % NDG]
                xT = xTs[b % 2]

                for n in range(2):
                    tp = psp.tile([D, 128], fp, tag="xt", name="tp")
                    nc.tensor.transpose(tp[:], xall[:, b, n, :], ident)
                    nc.scalar.copy(xT[:, 1 + n * 128 : 1 + (n + 1) * 128], tp[:])

                # conv: h[h, t] = sum_k w_tap[k].T @ xT shifted by k
                hps = psp.tile([H, T], fp, tag="hps")
                for k in range(KW):
                    nc.tensor.matmul(
                        hps[:], w_tap[k][:], xT[:, k : k + T],
                        start=(k == 0), stop=(k == KW - 1),
                    )
                h1 = work.tile([H, T], fp, tag="h1")
                nc.scalar.activation(h1[:], hps[:], AF.Relu, bias=cb_sb)

                mm1 = psp.tile([H, T], fp, tag="mm")
                nc.tensor.matmul(mm1[:], w1_sb, h1[:], start=True, stop=True)
                h2 = work.tile([H, T], fp, tag="h2")
                nc.scalar.activation(h2[:], mm1[:], AF.Relu, bias=b1_sb)

                mm2 = psp.tile([H, T], fp, tag="mm2")
                nc.tensor.matmul(mm2[:], w2_sb, h2[:], start=True, stop=True)
                h3 = work.tile([H, T], fp, tag="h3")
                nc.scalar.activation(h3[:], mm2[:], AF.Relu, bias=b2_sb)

                m16 = psp.tile([L, T], fp, tag="m16")
                nc.tensor.matmul(m16[:], w3_sb[:, 0:L], h3[:], start=True, stop=True)
                p32 = psp.tile([2 * L, T], fp, tag="p32")
                nc.tensor.matmul(
                    p32[:], w3_sb[:, L : 3 * L], h3[:], start=True, stop=True
                )

                nc.scalar.activation(
                    mean_all[:, b * T : (b + 1) * T], m16[:],
                    AF.Identity, bias=b3m_sb,
                )

                # prec pre-activations, bias added; then pick cols t = l*16+j
                # (j < 8) and transpose to [128, 32]: partition (l, j), free c
                praw = work.tile([2 * L, T], fp, tag="praw")
                nc.scalar.activation(praw[:], p32[:], AF.Identity, bias=b3p_sb)
                pack = work.tile([2 * L, 128], fp, tag="pack")
                last_pack = nc.scalar.copy(
                    pack[:], praw.rearrange("c (l r) -> c r l", r=16)[:, 0:8, :]
                )
                rv_ps = psp.tile([128, 2 * L], fp, tag="rv")
                nc.tensor.transpose(rv_ps[:], pack[:], ident[0:32, 0:32])

                # rv2 = 1 / (softplus(rv) + 1)
                # softplus(x)+1 = ln(e*exp(x) + e), fused into the Ln op
                ex = work.tile([128, 2 * L], fp, tag="ex")
                nc.scalar.activation(ex[:], rv_ps[:], AF.Exp)
                sp1 = work.tile([128, 2 * L], fp, tag="sp1")
                nc.scalar.activation(sp1[:], ex[:], AF.Ln, bias=e_const, scale=e_const)
                rv2 = work.tile([128, 2 * L], fp, tag="rv2")
                recip = nc.vector.reciprocal(rv2[:], sp1[:])
                if prev_diag is not None:
                    # keep DVE program order == batch order so obs_self's
                    # self-wait covers every earlier DVE tick
                    add_dep_helper(recip.ins, prev_diag.ins, sync=False,
                                   reason="DVE cross-batch order")

                # Wait-splitting chain (one semaphore wait max per inst):
                # obs_self absorbs the DVE pipeline wait on rv2 (covering
                # all earlier DVE ticks); obs_buf (ordered after it)
                # absorbs the DMAHW WAR wait from the previous scale-DMA
                # read of buf; the diag-slot copy then has its deps
                # observed except the adjacent same-engine WAW.
                dve_obs = work.tile([128, 2 * L], fp, tag="dve_obs", bufs=1)
                obs_self = nc.vector.tensor_copy(dve_obs[:], rv2[:])
                obs_buf = nc.vector.memset(buf[:, 1000:1001], 0.0)
                add_dep_helper(obs_buf.ins, obs_self.ins, sync=False,
                               reason="order DVE observer chain")
                # write diag slots: buf[p, 224 + 257c] = rv2[p, c]
                prev_diag = nc.vector.tensor_copy(
                    buf[:, 224 : 224 + 31 * 257 + 1 : 257], rv2[:]
                )
                # Stream out: 8 DMAs per batch, one per stripe j
                # (16 contiguous partitions each); the column window
                # [224-32j, 224-32j+8192) absorbs the shift.
                for j in range(8):
                    off = 224 - 32 * j
                    eng = nc.sync if j % 2 == 0 else nc.scalar
                    eng.dma_start(
                        scale_v[b, j],
                        buf[16 * j : 16 * (j + 1), off : off + 8192],
                    )

            nc.sync.dma_start(
                mean_o.rearrange("b l t -> l b t"),
                mean_all.rearrange("l (b t) -> l b t", b=BPC),
            )

    # Force Exp and Ln onto the single combined act-func set so only one
    # LoadActFuncSet is ever needed (the default chooser alternates between
    # an Exp-only and an Ln-only set -> 2 table swaps per batch).
    import concourse.bacc as bacc_mod
    orig_tables = bacc_mod.get_activation_tables

    def _patched_tables(arch):
        tabs = dict(orig_tables(arch))
        used = {mybir.ActivationFunctionType.Exp,
                mybir.ActivationFunctionType.Ln,
                mybir.ActivationFunctionType.Relu,
                mybir.ActivationFunctionType.Identity,
                mybir.ActivationFunctionType.Copy}
        both = {k for k, v in tabs.items() if used <= v}
        if both:
            for k in list(tabs):
                if k not in both:
                    tabs[k] = tabs[k] - used
        return tabs

    bacc_mod.get_activation_tables = _patched_tables
    try:
        nc.compile()
    finally:
        bacc_mod.get_activation_tables = orig_tables
    return nc


def _get_nc():
    if "nc" not in _CACHE:
        _CACHE["nc"] = _build_bass()
    return _CACHE["nc"]


def kernel(**inputs):
    from concourse.bass_utils import run_bass_kernel_spmd

    arrs = {k: np.ascontiguousarray(np.asarray(v), dtype=np.float32)
            for k, v in inputs.items()}
    x = arrs["x"]
    shared = {k: arrs[k] for k in
              ("conv_w", "conv_b", "w1", "b1", "w2", "b2", "w3", "b3")}
    in_maps = [
        {"x": np.ascontiguousarray(x[c * BPC : (c + 1) * BPC]), **shared}
        for c in range(NCORES)
    ]
    res = run_bass_kernel_spmd(_get_nc(), in_maps, core_ids=list(range(NCORES)))
    mean = np.concatenate([r["mean_o"] for r in res.results], axis=0)
    scale = np.concatenate([r["scale_o"] for r in res.results], axis=0)
    return mean, scale
